# revision 4
# baseline (speedup 1.0000x reference)
"""Trainium2 Bass kernel for CausalSelfAttention variant (B=4, N=2048, D=1024, H=16, dk=dv=64).

Reference quirks faithfully implemented:
  - softmax over axis=2 (query axis): A[:, j] normalized over i (column softmax)
  - no 1/sqrt(dk) scaling
  - raw reshape (B,H,N,dv) -> (B,N,H*dv): output rows g*128:(g+1)*128 of batch b
    depend only on head g: out[b, g*128+r, :] = AV[b,g].reshape(128,1024)[r] @ W_O

Sharding (8 cores): core c handles batch b=c//2, global heads (c%2)*8 .. +8.
Each core produces out[b, (c%2)*1024 : +1024, :].

Math per (core, head g):
  QT/KT [64, 2048] (k-major) via W-stationary matmuls, heads packed in pairs
  (gA rows 0:64 / gB rows 64:128 of a [128, 2048] tile) so S^T uses row-tiled
  K=64 matmuls. V [j, dv] via X^T-stationary matmuls, 4 heads packed (N=256).
  S^T[j, i] = K Q^T, exp via ScalarE with accum_out giving column sums c[j]
  for free, Vtil = V/c, AV^T[dv, i] = sum_j Vtil[j,dv] E^T[j,i], rearranged
  via identity matmuls into avt2[64*(m%2)+dv, 128*(m//2)+r], then
  O = avt2-tiles^T @ W_O-tiles (K=128 full-util matmuls).

All matmul operands are float32r (TF32-like, 1 cyc/row at N>=256).
"""

import numpy as np
from contextlib import ExitStack

import concourse.bass as bass
import concourse.tile as tile
from concourse import bacc, mybir
from concourse.bass_utils import run_bass_kernel_spmd

B, N, D, H, DK, DV = 4, 2048, 1024, 16, 64, 64
NCORES = 8
HPC = 8          # heads per core
F32 = mybir.dt.float32
F32R = mybir.dt.float32r
EXP = mybir.ActivationFunctionType.Exp

_compiled = None


def _identity_np():
    # ID[p, 192*par + p%64] = 1 : lhsT tiles for the AV^T -> avt2 rearrangement
    ident = np.zeros((128, 256), np.float32)
    for p in range(128):
        for par in range(2):
            ident[p, 192 * par + (p % 64)] = 1.0
    return ident


def _build():
    nc = bacc.Bacc("TRN2", target_bir_lowering=False, debug=False,
                   num_devices=NCORES)
    x_d = nc.dram_tensor("X", [N, D], F32, kind="ExternalInput").ap()
    wq_d = nc.dram_tensor("WQ", [HPC, D, DK], F32, kind="ExternalInput").ap()
    wk_d = nc.dram_tensor("WK", [HPC, D, DK], F32, kind="ExternalInput").ap()
    wv_d = nc.dram_tensor("WV", [HPC, D, DV], F32, kind="ExternalInput").ap()
    wo_d = nc.dram_tensor("WO", [D, D], F32, kind="ExternalInput").ap()
    id_d = nc.dram_tensor("ID", [128, 256], F32, kind="ExternalInput").ap()
    out_d = nc.dram_tensor("OUT", [HPC * 128, D], F32, kind="ExternalOutput").ap()

    with tile.TileContext(nc) as tc:
        with ExitStack() as ctx:
            persist = ctx.enter_context(tc.tile_pool(name="persist", bufs=1))
            stage = ctx.enter_context(tc.tile_pool(name="stage", bufs=1))
            wpool = ctx.enter_context(tc.tile_pool(name="w", bufs=1))
            vpool = ctx.enter_context(tc.tile_pool(name="v", bufs=1))
            qkpool = ctx.enter_context(tc.tile_pool(name="qk", bufs=1))
            epool = ctx.enter_context(tc.tile_pool(name="e", bufs=3))
            avtpool = ctx.enter_context(tc.tile_pool(name="avt", bufs=1))
            a2pool = ctx.enter_context(tc.tile_pool(name="a2", bufs=2))
            opool = ctx.enter_context(tc.tile_pool(name="o", bufs=2))
            small = ctx.enter_context(tc.tile_pool(name="sm", bufs=8))
            zpool = ctx.enter_context(tc.tile_pool(name="z", bufs=1))
            psp = ctx.enter_context(tc.tile_pool(name="ps", bufs=2, space="PSUM"))
            avp = ctx.enter_context(tc.tile_pool(name="avp", bufs=1, space="PSUM"))

            # ---- one-time init ----
            xtr = persist.tile([128, 8, N], F32R, tag="xtr")     # X^T [d, i]
            wo2 = persist.tile([128, 8, D], F32R, tag="wo2")     # WO[128k+p, :]
            idr = persist.tile([128, 256], F32R, tag="idr")

            for k in range(8):
                st = stage.tile([128, N], F32, tag="stg")
                nc.sync.dma_start(st[:, 0:D], wo_d[k * 128:(k + 1) * 128, :])
                nc.vector.tensor_copy(wo2[:, k, :], st[:, 0:D])
            for td in range(8):
                st = stage.tile([128, N], F32, tag="stg")
                for half in range(2):
                    c0 = td * 128 + half * 64
                    nc.sync.dma_start_transpose(
                        st[half * 64:(half + 1) * 64, :], x_d[:, c0:c0 + 64])
                nc.vector.tensor_copy(xtr[:, td, :], st[:])
            st = stage.tile([128, N], F32, tag="stg")
            nc.sync.dma_start(st[:, 0:256], id_d[:])
            nc.vector.tensor_copy(idr[:], st[:, 0:256])
            z32 = zpool.tile([128, 512], F32, tag="z32")
            nc.vector.memset(z32[:], 0.0)
            zl = zpool.tile([1, 128], F32R, tag="zl")
            zr = zpool.tile([1, 512], F32R, tag="zr")
            nc.vector.tensor_copy(zl[:], z32[0:1, 0:128])
            nc.vector.tensor_copy(zr[:], z32[0:1, :])
            # persistent [128, 128] Vtil tiles; the unused half stays zero so a
            # full-M matmul adds zero rows instead of needing col tile_position
            vt2 = []
            for hh in range(2):
                t = zpool.tile([128, 128], F32R, tag=f"vt2_{hh}")
                nc.vector.tensor_copy(t[:, 0:128], z32[:, 0:128])
                vt2.append(t)

            for p2 in range(2):             # groups of 4 heads
                # V projection for heads 4*p2 .. 4*p2+3, packed N=256
                wv4 = wpool.tile([128, 8, 256], F32R, tag="wv4")
                st = stage.tile([128, N], F32, tag="stg")
                stv = st[:].rearrange("p (t k) -> p t k", k=256)
                for h4 in range(4):
                    g = 4 * p2 + h4
                    nc.sync.dma_start(
                        stv[:, :, h4 * 64:(h4 + 1) * 64],
                        wv_d[g].rearrange("(t p) k -> p t k", p=128))
                nc.vector.tensor_copy(wv4[:], stv)
                v4 = vpool.tile([128, 16, 256], F32R, tag="v4")
                for jt in range(16):
                    psv = psp.tile([128, 1024], F32, tag="ps")
                    for d in range(8):
                        nc.tensor.matmul(psv[:, 0:256],
                                         xtr[:, d, jt * 128:(jt + 1) * 128],
                                         wv4[:, d, :],
                                         start=(d == 0), stop=(d == 7))
                    nc.vector.tensor_copy(v4[:, jt, :], psv[:, 0:256])

                for pp in range(2):         # head pairs within the group
                    pair = 2 * p2 + pp
                    # -- load + round packed W for the pair --
                    wq = wpool.tile([128, 8, 128], F32R, tag="wq")
                    wk = wpool.tile([128, 8, 128], F32R, tag="wk")
                    for wt, wsrc in ((wq, wq_d), (wk, wk_d)):
                        st = stage.tile([128, N], F32, tag="stg")
                        stw = st[:, 0:1024].rearrange("p (t k) -> p t k", k=128)
                        for hh in range(2):
                            g = 2 * pair + hh
                            nc.sync.dma_start(
                                stw[:, :, hh * 64:(hh + 1) * 64],
                                wsrc[g].rearrange("(t p) k -> p t k", p=128))
                        nc.vector.tensor_copy(wt[:], stw)

                    # -- QK projections: QT/KT packed [128, 2048] --
                    qt2 = qkpool.tile([128, N], F32R, tag="qt")
                    kt2 = qkpool.tile([128, N], F32R, tag="kt")
                    for dst, wt in ((qt2, wq), (kt2, wk)):
                        for ib in range(4):
                            ps = psp.tile([128, 1024], F32, tag="ps")
                            for d in range(8):
                                nc.tensor.matmul(
                                    ps[:, 0:512], wt[:, d, :],
                                    xtr[:, d, ib * 512:(ib + 1) * 512],
                                    start=(d == 0), stop=(d == 7))
                            nc.vector.tensor_copy(dst[:, ib * 512:(ib + 1) * 512],
                                                  ps[:, 0:512])

                    # -- attention --
                    av = avp.tile([128, N], F32, tag="av")
                    for c4 in range(4):   # zero-fill: set has_written everywhere
                        nc.tensor.matmul(av[:, c4 * 512:(c4 + 1) * 512],
                                         zl[:], zr[:], start=True, stop=False)
                    for jt in range(16):
                        for hh in range(2):
                            base = hh * 64
                            h4 = 2 * pp + hh
                            ss = []
                            for ihalf in range(2):
                                s = psp.tile([128, 1024], F32, tag="ps")
                                for c2 in range(2):
                                    i0 = ihalf * 1024 + c2 * 512
                                    nc.tensor.matmul(
                                        s[:, c2 * 512:(c2 + 1) * 512],
                                        kt2[base:base + 64, jt * 128:(jt + 1) * 128],
                                        qt2[base:base + 64, i0:i0 + 512],
                                        start=True, stop=True,
                                        tile_position=(base, 0))
                                ss.append(s)
                            accs = []
                            es = []
                            for ihalf in range(2):
                                e = epool.tile([128, 1024], F32R, tag="e")
                                a = small.tile([128, 1], F32, tag=f"acc{ihalf}")
                                nc.scalar.activation(e[:], ss[ihalf][:], EXP,
                                                     accum_out=a[:])
                                es.append(e)
                                accs.append(a)
                            cs = small.tile([128, 1], F32, tag="c")
                            nc.vector.tensor_add(cs[:], accs[0][:], accs[1][:])
                            rc = small.tile([128, 1], F32, tag="rc")
                            nc.vector.reciprocal(rc[:], cs[:])
                            nc.vector.tensor_scalar_mul(
                                vt2[hh][:, base:base + 64],
                                v4[:, jt, h4 * 64:(h4 + 1) * 64], rc[:])
                            for c4 in range(4):
                                nc.tensor.matmul(
                                    av[:, c4 * 512:(c4 + 1) * 512],
                                    vt2[hh][:],
                                    es[c4 // 2][:, (c4 % 2) * 512:(c4 % 2) * 512 + 512],
                                    start=False, stop=(jt == 15))

                    # -- O projection --
                    avt = avtpool.tile([128, N], F32R, tag="avt")
                    nc.vector.tensor_copy(avt[:], av[:])
                    for hh in range(2):
                        base = hh * 64
                        a2ps = psp.tile([128, 1024], F32, tag="ps")
                        for c2 in range(2):   # zero-fill the two banks
                            nc.tensor.matmul(a2ps[:, c2 * 512:(c2 + 1) * 512],
                                             zl[:], zr[:], start=True, stop=False)
                        av_v = avt[base:base + 64, :].rearrange(
                            "p (r m) -> p m r", m=16)
                        for par in range(2):
                            for c2 in range(2):
                                m0 = 8 * c2 + par
                                nc.tensor.matmul(
                                    a2ps[:, c2 * 512:(c2 + 1) * 512],
                                    idr[base:base + 64, 128 * par:128 * par + 128],
                                    av_v[:, m0:m0 + 7:2, :],
                                    start=False, stop=(par == 1),
                                    tile_position=(base, 0))
                        avt2 = a2pool.tile([128, 1024], F32R, tag="a2")
                        nc.vector.tensor_copy(avt2[:], a2ps[:])
                        pso = psp.tile([128, 1024], F32, tag="ps")
                        for dblk in range(2):
                            for k in range(8):
                                nc.tensor.matmul(
                                    pso[:, dblk * 512:(dblk + 1) * 512],
                                    avt2[:, k * 128:(k + 1) * 128],
                                    wo2[:, k, dblk * 512:(dblk + 1) * 512],
                                    start=(k == 0), stop=(k == 7))
                        o_sb = opool.tile([128, D], F32, tag="o")
                        nc.vector.tensor_copy(o_sb[:], pso[:])
                        g = 2 * pair + hh
                        nc.sync.dma_start(out_d[g * 128:(g + 1) * 128, :], o_sb[:])

    nc.compile()
    return nc


def _get_compiled():
    global _compiled
    if _compiled is None:
        _compiled = _build()
    return _compiled


def _run(in_maps, **kwargs):
    nc = _get_compiled()
    return run_bass_kernel_spmd(nc, in_maps, core_ids=list(range(NCORES)),
                                **kwargs)


def _make_in_maps(inputs):
    X = np.ascontiguousarray(np.asarray(inputs["X"], dtype=np.float32))
    WQ = np.ascontiguousarray(np.asarray(inputs["W_Q"], dtype=np.float32))
    WK = np.ascontiguousarray(np.asarray(inputs["W_K"], dtype=np.float32))
    WV = np.ascontiguousarray(np.asarray(inputs["W_V"], dtype=np.float32))
    WO = np.ascontiguousarray(np.asarray(inputs["W_O"], dtype=np.float32))
    ident = _identity_np()
    in_maps = []
    for c in range(NCORES):
        b = c // 2
        hs = (c % 2) * HPC
        in_maps.append({
            "X": X[b],
            "WQ": np.ascontiguousarray(WQ[hs:hs + HPC]),
            "WK": np.ascontiguousarray(WK[hs:hs + HPC]),
            "WV": np.ascontiguousarray(WV[hs:hs + HPC]),
            "WO": WO,
            "ID": ident,
        })
    return in_maps


def _assemble(results):
    out = np.empty((B, N, D), np.float32)
    for c in range(NCORES):
        b = c // 2
        r0 = (c % 2) * HPC * 128
        out[b, r0:r0 + HPC * 128, :] = results[c]["OUT"]
    return out


def kernel(**inputs) -> np.ndarray:
    res = _run(_make_in_maps(inputs))
    return _assemble(res.results)


def kernel_profiled(inputs):
    """Returns (output, BassKernelResults-with-trace) for test harnesses."""
    res = _run(_make_in_maps(inputs), trace=True)
    return _assemble(res.results), res


# revision 6
# speedup vs baseline: 2.9642x; 2.9642x over previous
"""Trainium2 Bass kernel for CausalSelfAttention variant (B=4, N=2048, D=1024, H=16, dk=dv=64).

Reference quirks faithfully implemented:
  - softmax over axis=2 (query axis): A[:, j] normalized over i (column softmax)
  - no 1/sqrt(dk) scaling
  - raw reshape (B,H,N,dv) -> (B,N,H*dv): output rows g*128:(g+1)*128 of batch b
    depend only on head g: out[b, g*128+r, :] = AV[b,g].reshape(128,1024)[r] @ W_O

Sharding (8 cores): core c handles batch b=c//2, global heads (c%2)*8 .. +8.
Each core produces out[b, (c%2)*1024 : +1024, :].

Per core/head:
  X^T built on-chip via PE transposes (bf16). QT/KT [64, 2048] via W-stationary
  matmuls with head pairs packed (gA rows 0:64 / gB rows 64:128) so S^T runs as
  row-tiled K=64 matmuls. V [j, dv] via X^T-stationary matmuls, 4 heads packed.
  S^T = K Q^T into fp32 PSUM, exp on ScalarE (accum_out gives the column sums
  free), Vtil = V/c zero-padded to [128, 128] so AV accumulates full-partition,
  AV^T rearranged via identity matmuls into avt2[64*(m%2)+dv, 128*(m//2)+r],
  then O = avt2-tiles^T @ W_O-tiles (K=128 matmuls).

All matmul operands are bf16 (fp32 PSUM accumulation); softmax stats stay fp32.
"""

import numpy as np
from contextlib import ExitStack

import concourse.bass as bass
import concourse.tile as tile
from concourse import bacc, mybir
from concourse.bass_utils import run_bass_kernel_spmd

B, N, D, H, DK, DV = 4, 2048, 1024, 16, 64, 64
NCORES = 8
HPC = 8          # heads per core
F32 = mybir.dt.float32
BF16 = mybir.dt.bfloat16
EXP = mybir.ActivationFunctionType.Exp

_compiled = None


def _identity_np():
    # cols 0:256 : ID[p, 192*par + p%64] = 1  (avt -> avt2 rearrangement lhsT)
    # cols 256:384: plain I128 (PE-transpose identity)
    ident = np.zeros((128, 384), np.float32)
    for p in range(128):
        for par in range(2):
            ident[p, 192 * par + (p % 64)] = 1.0
        ident[p, 256 + p] = 1.0
    return ident


def _build():
    nc = bacc.Bacc("TRN2", target_bir_lowering=False, debug=False,
                   num_devices=NCORES)
    x_d = nc.dram_tensor("X", [N, D], F32, kind="ExternalInput").ap()
    wq_d = nc.dram_tensor("WQ", [HPC, D, DK], F32, kind="ExternalInput").ap()
    wk_d = nc.dram_tensor("WK", [HPC, D, DK], F32, kind="ExternalInput").ap()
    wv_d = nc.dram_tensor("WV", [HPC, D, DV], F32, kind="ExternalInput").ap()
    wo_d = nc.dram_tensor("WO", [D, D], F32, kind="ExternalInput").ap()
    id_d = nc.dram_tensor("ID", [128, 384], F32, kind="ExternalInput").ap()
    out_d = nc.dram_tensor("OUT", [HPC * 128, D], F32, kind="ExternalOutput").ap()

    with tile.TileContext(nc) as tc:
        with ExitStack() as ctx:
            persist = ctx.enter_context(tc.tile_pool(name="persist", bufs=1))
            stage = ctx.enter_context(tc.tile_pool(name="stage", bufs=2))
            wpool = ctx.enter_context(tc.tile_pool(name="w", bufs=2))
            vpool = ctx.enter_context(tc.tile_pool(name="v", bufs=2))
            qkpool = ctx.enter_context(tc.tile_pool(name="qk", bufs=2))
            epool = ctx.enter_context(tc.tile_pool(name="e", bufs=6))
            avtpool = ctx.enter_context(tc.tile_pool(name="avt", bufs=2))
            a2pool = ctx.enter_context(tc.tile_pool(name="a2", bufs=2))
            opool = ctx.enter_context(tc.tile_pool(name="o", bufs=2))
            small = ctx.enter_context(tc.tile_pool(name="sm", bufs=8))
            zpool = ctx.enter_context(tc.tile_pool(name="z", bufs=1))
            psp = ctx.enter_context(tc.tile_pool(name="ps", bufs=2, space="PSUM"))
            avp = ctx.enter_context(tc.tile_pool(name="avp", bufs=1, space="PSUM"))

            # ---- one-time init ----
            xtr = persist.tile([128, 8, N], BF16, tag="xtr")     # X^T [d, i]
            wo2 = persist.tile([128, 8, D], BF16, tag="wo2")     # WO[128k+p, :]
            idr = persist.tile([128, 384], BF16, tag="idr")

            st = stage.tile([128, 2048], F32, tag="stg")
            nc.sync.dma_start(st[:, 0:384], id_d[:])
            nc.vector.tensor_copy(idr[:], st[:, 0:384])
            idT = idr[:, 256:384]

            for k in range(8):
                st = stage.tile([128, 2048], F32, tag="stg")
                nc.sync.dma_start(st[:, 0:D], wo_d[k * 128:(k + 1) * 128, :])
                nc.vector.tensor_copy(wo2[:, k, :], st[:, 0:D])

            # X -> bf16 -> PE-transpose into xtr
            for it in range(16):
                st = stage.tile([128, 2048], F32, tag="stg")
                nc.sync.dma_start(st[:, 0:D], x_d[it * 128:(it + 1) * 128, :])
                stb = stage.tile([128, D], BF16, tag="stb")
                nc.vector.tensor_copy(stb[:], st[:, 0:D])
                for td in range(8):
                    pst = psp.tile([128, 128], BF16, tag="ps")
                    nc.tensor.transpose(pst[:], stb[:, td * 128:(td + 1) * 128],
                                        idT)
                    nc.vector.tensor_copy(
                        xtr[:, td, it * 128:(it + 1) * 128], pst[:])

            z32 = zpool.tile([128, 512], F32, tag="z32")
            nc.vector.memset(z32[:], 0.0)
            zl = zpool.tile([1, 128], BF16, tag="zl")
            zr = zpool.tile([1, 512], BF16, tag="zr")
            nc.vector.tensor_copy(zl[:], z32[0:1, 0:128])
            nc.vector.tensor_copy(zr[:], z32[0:1, :])
            # persistent [128, 128] Vtil tiles; the unused half stays zero so a
            # full-M matmul adds zero rows to the other head's AV partitions
            vt2 = []
            for hh in range(2):
                t = zpool.tile([128, 128], BF16, tag=f"vt2_{hh}")
                nc.vector.tensor_copy(t[:, 0:128], z32[:, 0:128])
                vt2.append(t)

            for p2 in range(2):             # groups of 4 heads
                # V projection for heads 4*p2 .. 4*p2+3, packed N=256
                wv4 = wpool.tile([128, 8, 256], BF16, tag="wv4")
                st = stage.tile([128, 2048], F32, tag="stg")
                stv = st[:].rearrange("p (t k) -> p t k", k=256)
                for h4 in range(4):
                    g = 4 * p2 + h4
                    nc.sync.dma_start(
                        stv[:, :, h4 * 64:(h4 + 1) * 64],
                        wv_d[g].rearrange("(t p) k -> p t k", p=128))
                nc.vector.tensor_copy(wv4[:], stv)
                v4 = vpool.tile([128, 16, 256], BF16, tag="v4")
                for jt in range(16):
                    psv = psp.tile([128, 1024], F32, tag="ps")
                    for d in range(8):
                        nc.tensor.matmul(psv[:, 0:256],
                                         xtr[:, d, jt * 128:(jt + 1) * 128],
                                         wv4[:, d, :],
                                         start=(d == 0), stop=(d == 7))
                    nc.vector.tensor_copy(v4[:, jt, :], psv[:, 0:256])

                for pp in range(2):         # head pairs within the group
                    pair = 2 * p2 + pp
                    # -- load packed W for the pair --
                    wq = wpool.tile([128, 8, 128], BF16, tag="wq")
                    wk = wpool.tile([128, 8, 128], BF16, tag="wk")
                    for wt, wsrc in ((wq, wq_d), (wk, wk_d)):
                        st = stage.tile([128, 2048], F32, tag="stg")
                        stw = st[:, 0:1024].rearrange("p (t k) -> p t k", k=128)
                        for hh in range(2):
                            g = 2 * pair + hh
                            nc.sync.dma_start(
                                stw[:, :, hh * 64:(hh + 1) * 64],
                                wsrc[g].rearrange("(t p) k -> p t k", p=128))
                        nc.vector.tensor_copy(wt[:], stw)

                    # -- QK projections: QT/KT packed [128, 2048] --
                    qt2 = qkpool.tile([128, N], BF16, tag="qt")
                    kt2 = qkpool.tile([128, N], BF16, tag="kt")
                    for dst, wt in ((qt2, wq), (kt2, wk)):
                        for ib in range(4):
                            ps = psp.tile([128, 1024], F32, tag="ps")
                            for d in range(8):
                                nc.tensor.matmul(
                                    ps[:, 0:512], wt[:, d, :],
                                    xtr[:, d, ib * 512:(ib + 1) * 512],
                                    start=(d == 0), stop=(d == 7))
                            nc.vector.tensor_copy(dst[:, ib * 512:(ib + 1) * 512],
                                                  ps[:, 0:512])

                    # -- attention --
                    av = avp.tile([128, N], F32, tag="av")
                    for c4 in range(4):   # zero-fill: set has_written everywhere
                        nc.tensor.matmul(av[:, c4 * 512:(c4 + 1) * 512],
                                         zl[:], zr[:], start=True, stop=False)
                    for jt in range(16):
                        for hh in range(2):
                            base = hh * 64
                            h4 = 2 * pp + hh
                            ss = []
                            for ihalf in range(2):
                                s = psp.tile([128, 1024], F32, tag="ps")
                                for c2 in range(2):
                                    i0 = ihalf * 1024 + c2 * 512
                                    nc.tensor.matmul(
                                        s[:, c2 * 512:(c2 + 1) * 512],
                                        kt2[base:base + 64, jt * 128:(jt + 1) * 128],
                                        qt2[base:base + 64, i0:i0 + 512],
                                        start=True, stop=True,
                                        tile_position=(base, 0))
                                ss.append(s)
                            accs = []
                            es = []
                            for ihalf in range(2):
                                e = epool.tile([128, 1024], BF16, tag="e")
                                a = small.tile([128, 1], F32, tag=f"acc{ihalf}")
                                nc.scalar.activation(e[:], ss[ihalf][:], EXP,
                                                     accum_out=a[:])
                                es.append(e)
                                accs.append(a)
                            cs = small.tile([128, 1], F32, tag="c")
                            nc.vector.tensor_add(cs[:], accs[0][:], accs[1][:])
                            rc = small.tile([128, 1], F32, tag="rc")
                            nc.vector.reciprocal(rc[:], cs[:])
                            nc.vector.tensor_scalar_mul(
                                vt2[hh][:, base:base + 64],
                                v4[:, jt, h4 * 64:(h4 + 1) * 64], rc[:])
                            for c4 in range(4):
                                nc.tensor.matmul(
                                    av[:, c4 * 512:(c4 + 1) * 512],
                                    vt2[hh][:],
                                    es[c4 // 2][:, (c4 % 2) * 512:(c4 % 2) * 512 + 512],
                                    start=False, stop=(jt == 15))

                    # -- O projection --
                    avt = avtpool.tile([128, N], BF16, tag="avt")
                    nc.vector.tensor_copy(avt[:], av[:])
                    for hh in range(2):
                        base = hh * 64
                        a2ps = psp.tile([128, 1024], F32, tag="ps")
                        for c2 in range(2):   # zero-fill the two banks
                            nc.tensor.matmul(a2ps[:, c2 * 512:(c2 + 1) * 512],
                                             zl[:], zr[:], start=True, stop=False)
                        av_v = avt[base:base + 64, :].rearrange(
                            "p (r m) -> p m r", m=16)
                        for par in range(2):
                            for c2 in range(2):
                                m0 = 8 * c2 + par
                                nc.tensor.matmul(
                                    a2ps[:, c2 * 512:(c2 + 1) * 512],
                                    idr[base:base + 64, 128 * par:128 * par + 128],
                                    av_v[:, m0:m0 + 7:2, :],
                                    start=False, stop=(par == 1),
                                    tile_position=(base, 0))
                        avt2 = a2pool.tile([128, 1024], BF16, tag="a2")
                        nc.vector.tensor_copy(avt2[:], a2ps[:])
                        pso = psp.tile([128, 1024], F32, tag="ps")
                        for dblk in range(2):
                            for k in range(8):
                                nc.tensor.matmul(
                                    pso[:, dblk * 512:(dblk + 1) * 512],
                                    avt2[:, k * 128:(k + 1) * 128],
                                    wo2[:, k, dblk * 512:(dblk + 1) * 512],
                                    start=(k == 0), stop=(k == 7))
                        o_sb = opool.tile([128, D], F32, tag="o")
                        nc.vector.tensor_copy(o_sb[:], pso[:])
                        g = 2 * pair + hh
                        nc.sync.dma_start(out_d[g * 128:(g + 1) * 128, :], o_sb[:])

    nc.compile()
    return nc


def _get_compiled():
    global _compiled
    if _compiled is None:
        _compiled = _build()
    return _compiled


def _run(in_maps, **kwargs):
    nc = _get_compiled()
    return run_bass_kernel_spmd(nc, in_maps, core_ids=list(range(NCORES)),
                                **kwargs)


def _make_in_maps(inputs):
    X = np.ascontiguousarray(np.asarray(inputs["X"], dtype=np.float32))
    WQ = np.ascontiguousarray(np.asarray(inputs["W_Q"], dtype=np.float32))
    WK = np.ascontiguousarray(np.asarray(inputs["W_K"], dtype=np.float32))
    WV = np.ascontiguousarray(np.asarray(inputs["W_V"], dtype=np.float32))
    WO = np.ascontiguousarray(np.asarray(inputs["W_O"], dtype=np.float32))
    ident = _identity_np()
    in_maps = []
    for c in range(NCORES):
        b = c // 2
        hs = (c % 2) * HPC
        in_maps.append({
            "X": X[b],
            "WQ": np.ascontiguousarray(WQ[hs:hs + HPC]),
            "WK": np.ascontiguousarray(WK[hs:hs + HPC]),
            "WV": np.ascontiguousarray(WV[hs:hs + HPC]),
            "WO": WO,
            "ID": ident,
        })
    return in_maps


def _assemble(results):
    out = np.empty((B, N, D), np.float32)
    for c in range(NCORES):
        b = c // 2
        r0 = (c % 2) * HPC * 128
        out[b, r0:r0 + HPC * 128, :] = results[c]["OUT"]
    return out


def kernel(**inputs) -> np.ndarray:
    res = _run(_make_in_maps(inputs))
    return _assemble(res.results)


def kernel_profiled(inputs):
    """Returns (output, BassKernelResults-with-trace) for test harnesses."""
    res = _run(_make_in_maps(inputs), trace=True)
    return _assemble(res.results), res
